# revision 20
# baseline (speedup 1.0000x reference)
"""LIFSpike Trainium2 kernel (Bass/Tile), SPMD over 8 NeuronCores.

Reference semantics (T=4, tau=2, vth=1, vreset=0, decay_input=False,
hard reset):
    xs = x.reshape(T, B//T, C, H, W)
    v0 = 0
    h_t = v_t * 0.5 + x_t
    s_t = (h_t >= 1.0)
    v_{t+1} = h_t * (h_t < 1.0)
    out = s.reshape(B, C, H, W)

Kernel-side reformulation (exact in fp32 -- all rescalings are by powers
of two, which commute with fp rounding):
    r_t := 2^t * h_t,  host supplies x'_t = 2^t * x_t
    r_0     = x'_0                                   (plain DMA load)
    s_t     = (r_t >= 2^t)
    q_t     = (r_t < 2^t) * r_t   (= 2^t * v_{t+1})  (one STT op, DVE)
    r_{t+1} = q_t + x'_{t+1}

Engine assignment (DVE and the Pool engine contend ~3x on shared SBUF
ports, and fp32 matmul adds on PE measured 2-pass/too slow, so):
  * STT and the t1/t3 adds run on DVE (phase-ordered for pipelining).
  * The t2 adds ride on SWDGE accumulate-DMAs: the STT writes q into
    the t2 slab slice, then the DMA adds x'_2 from HBM on top.
  * The spike comparison runs on the Scalar (ACT) engine:
    sign(r_t + bias_t), bias_t = nextafter(-2^t, 0), fp8 out; the host
    decodes s = (value > 0).  Exact: r + bias == 0 only for
    r == 2^t*(1-2^-24), and sign(0)=0 decodes to s=0, correct.
  * Plain loads round-robin over BOTH HWDGE rings (sync + scalar):
    one ring sustains only ~312 GB/s; two together ~380+.

Host-side input layout per core (partition-major, t-major):
    x_core[p, t*8192 + b*2048 + j] = 2^t * x[t*32 + core*4 + b, flat=p*2048+j]
Output layout is b-major:
    s_core[p, b*8192 + t*2048 + j]
"""

import numpy as np

T = 4
BP = 32               # B // T
NCORES = 8
BPC = BP // NCORES    # chains per core = 4
SLICE = 256 * 32 * 32  # elements per (t, b) slice = 262144
P = 128
W = SLICE // P        # free elems per chain-timestep tile = 2048
FREE_T = BPC * W      # 8192 (one timestep slab, all chains)
FREE = T * FREE_T     # 32768
ACCUM_T = 2           # timestep whose x-add rides on accumulate-DMAs

_cache = {}


def _build_program():
    import concourse.bass as bass
    import concourse.tile as tile
    from concourse import bacc, mybir

    Alu = mybir.AluOpType
    Act = mybir.ActivationFunctionType
    f32 = mybir.dt.float32
    out_dt = mybir.dt.float8e4

    nc = bacc.Bacc(debug=False)
    x = nc.dram_tensor("x", [P, FREE], f32, kind="ExternalInput").ap()
    s = nc.dram_tensor("s", [P, FREE], out_dt, kind="ExternalOutput").ap()

    with tile.TileContext(nc) as tc:
        with (
            tc.tile_pool(name="state", bufs=1) as vpool,
            tc.tile_pool(name="sout", bufs=1) as spool,
        ):
            slabs = [
                vpool.tile([P, FREE_T], f32, tag=f"xs{t}", name=f"xs{t}")
                for t in range(T)
            ]
            qts = [
                vpool.tile([P, W], f32, tag=f"q{b}", name=f"q{b}")
                for b in range(BPC)
            ]
            outs = [
                spool.tile([P, T * W], out_dt, tag=f"s{b}", name=f"sout{b}")
                for b in range(BPC)
            ]
            biases = vpool.tile([P, T], f32, tag="bias", name="bias")
            for t in range(T):
                bias = float(np.nextafter(np.float32(-(1 << t)), np.float32(0)))
                nc.gpsimd.memset(biases[:, t:t + 1], bias)

            # plain loads, 1 MiB chunks, all on the sync HWDGE ring --
            # concurrent DMA on the scalar ring measured a ~20% slowdown of
            # every compute op.  t2 for chains 0/1 arrives via accum-DMAs;
            # load order tracks when each chunk is consumed.
            # t0 rides the scalar ring: it is idle during the ramp (before
            # any compute is running) so the dual-ring slowdown cannot bite,
            # and the sync ring starts on t1 immediately.
            for b in range(BPC):
                nc.scalar.dma_start(
                    slabs[0][:, b * W:(b + 1) * W], x[:, b * W:(b + 1) * W]
                )
            load_order = (
                [(1, b) for b in range(BPC)]
                + [(2, 3), (3, 3), (3, 0), (3, 1), (3, 2)]
            )
            for t, b in load_order:
                lo = t * FREE_T + b * W
                nc.sync.dma_start(
                    slabs[t][:, b * W:(b + 1) * W], x[:, lo:lo + W]
                )

            def state(b, t):
                return slabs[t][:, b * W:(b + 1) * W]

            def sign(b, t):
                nc.scalar.activation(
                    outs[b][:, t * W:(t + 1) * W], state(b, t), Act.Sign,
                    bias=biases[:, t:t + 1],
                )

            def store(b, t0_, t1_):
                nc.sync.dma_start(
                    s[:, b * T * W + t0_ * W:b * T * W + t1_ * W],
                    outs[b][:, t0_ * W:t1_ * W],
                )

            # prefetch the ACT Sign table during the load ramp
            nc.scalar.activation(
                outs[0][:, :1], biases[:, :1], Act.Sign, bias=biases[:, :1]
            )

            # t0: spikes + phase-batched STTs (arrival-paced)
            for b in range(BPC):
                sign(b, 0)
            for b in range(BPC):
                nc.vector.scalar_tensor_tensor(
                    qts[b][:], state(b, 0), 1.0, state(b, 0),
                    Alu.is_lt, Alu.mult,
                )
            # t1 per chain: [add(r1); STT(q1)].  Chains 0/1/2 then fire
            # their t2 accum-DMA immediately (13.5us+ completion latency
            # hides behind the rest of the DVE work); chain 3 does t2 on
            # DVE.
            for b in range(BPC):
                p = state(b, 1)
                nc.vector.tensor_tensor(p, p, qts[b][:], Alu.add)
                nxt = state(b, 2)
                if b < 3:
                    nc.vector.scalar_tensor_tensor(
                        nxt, p, 2.0, p, Alu.is_lt, Alu.mult
                    )
                    nc.gpsimd.dma_start(
                        nxt,
                        x[:, 2 * FREE_T + b * W:2 * FREE_T + (b + 1) * W],
                        accum_op=Alu.add,
                    )
                else:
                    nc.vector.scalar_tensor_tensor(
                        qts[b][:], p, 2.0, p, Alu.is_lt, Alu.mult
                    )
                sign(b, 1)
                store(b, 0, 2)
            # t2/t3: the DVE chain (3) first -- its adds run while the
            # accums complete -- then the accum chains in completion order
            for b in (3, 0, 1, 2):
                p = state(b, 2)
                if b == 3:
                    nc.vector.tensor_tensor(p, p, qts[b][:], Alu.add)
                sign(b, 2)
                store(b, 2, 3)
                nc.vector.scalar_tensor_tensor(
                    qts[b][:], p, 4.0, p, Alu.is_lt, Alu.mult
                )
                nxt = state(b, 3)
                nc.vector.tensor_tensor(nxt, nxt, qts[b][:], Alu.add)
                sign(b, 3)
                store(b, 3, 4)
    nc.compile()
    return nc


def _shard(x):
    # x: (128, 256, 32, 32) f32 -> list of 8 per-core [128, 32768] arrays,
    # timestep t pre-scaled by 2^t (exact in fp32)
    xr = np.ascontiguousarray(x).reshape(T, BP, SLICE)
    tscale = (2.0 ** np.arange(T, dtype=np.float32)).astype(np.float32)
    shards = []
    for k in range(NCORES):
        xk = xr[:, k * BPC:(k + 1) * BPC, :].reshape(T, BPC, P, W)
        xk = xk * tscale[:, None, None, None]
        xk = xk.transpose(2, 0, 1, 3).reshape(P, FREE)
        shards.append(np.asarray(xk, dtype=np.float32))
    return shards


def _unshard(parts):
    # parts: 8 per-core [128, 32768] arrays (fp8 sign values, b-major)
    # -> (128,256,32,32) f32 spikes; spike iff stored value > 0
    out = np.empty((T, BP, SLICE), dtype=np.float32)
    for k, sk in enumerate(parts):
        sk = (np.asarray(sk).astype(np.float32) > 0).astype(np.float32)
        sk = sk.reshape(P, BPC, T, W)
        out[:, k * BPC:(k + 1) * BPC, :] = (
            sk.transpose(2, 1, 0, 3).reshape(T, BPC, SLICE)
        )
    return out.reshape(T * BP, 256, 32, 32)


def _in_maps(x):
    return [{"x": sk} for sk in _shard(np.asarray(x, dtype=np.float32))]


def kernel(x):
    from concourse.bass_utils import run_bass_kernel_spmd

    if "nc" not in _cache:
        _cache["nc"] = _build_program()
    nc = _cache["nc"]

    res = run_bass_kernel_spmd(nc, _in_maps(x), list(range(NCORES)))
    return _unshard([res.results[k]["s"] for k in range(NCORES)])


# revision 24
# speedup vs baseline: 1.0889x; 1.0889x over previous
"""LIFSpike Trainium2 kernel (Bass/Tile), SPMD over 8 NeuronCores.

Reference semantics (T=4, tau=2, vth=1, vreset=0, decay_input=False,
hard reset):
    xs = x.reshape(T, B//T, C, H, W)
    v0 = 0
    h_t = v_t * 0.5 + x_t
    s_t = (h_t >= 1.0)
    v_{t+1} = h_t * (h_t < 1.0)
    out = s.reshape(B, C, H, W)

Kernel-side reformulation (exact in fp32 -- all rescalings are by powers
of two, which commute with fp rounding):
    r_t := 2^t * h_t,  host supplies x'_t = 2^t * x_t
    r_0     = x'_0                                   (plain DMA load)
    s_t     = (r_t >= 2^t)
    q_t     = (r_t < 2^t) * r_t   (= 2^t * v_{t+1})  (one STT op, DVE)
    r_{t+1} = q_t + x'_{t+1}

Engine assignment (DVE and the Pool engine contend ~3x on shared SBUF
ports, and fp32 matmul adds on PE measured 2-pass/too slow, so):
  * STT and the t1/t3 adds run on DVE (phase-ordered for pipelining).
  * The t2 adds ride on SWDGE accumulate-DMAs: the STT writes q into
    the t2 slab slice, then the DMA adds x'_2 from HBM on top.
  * The spike comparison runs on the Scalar (ACT) engine:
    sign(r_t + bias_t), bias_t = nextafter(-2^t, 0), fp8 out; the host
    decodes s = (value > 0).  Exact: r + bias == 0 only for
    r == 2^t*(1-2^-24), and sign(0)=0 decodes to s=0, correct.
  * Plain loads round-robin over BOTH HWDGE rings (sync + scalar):
    one ring sustains only ~312 GB/s; two together ~380+.

Host-side input layout per core (partition-major, t-major):
    x_core[p, t*8192 + b*2048 + j] = 2^t * x[t*32 + core*4 + b, flat=p*2048+j]
Output layout is b-major:
    s_core[p, b*8192 + t*2048 + j]
"""

import numpy as np

T = 4
BP = 32               # B // T
NCORES = 8
BPC = BP // NCORES    # chains per core = 4
SLICE = 256 * 32 * 32  # elements per (t, b) slice = 262144
P = 128
W = SLICE // P        # free elems per chain-timestep tile = 2048
FREE_T = BPC * W      # 8192 (one timestep slab, all chains)
FREE = T * FREE_T     # 32768
ACCUM_T = 2           # timestep whose x-add rides on accumulate-DMAs

_cache = {}


def _build_program():
    import concourse.bass as bass
    import concourse.tile as tile
    from concourse import bacc, mybir

    Alu = mybir.AluOpType
    Act = mybir.ActivationFunctionType
    f32 = mybir.dt.float32
    out_dt = mybir.dt.float8e4

    nc = bacc.Bacc(debug=False)
    x = nc.dram_tensor("x", [P, FREE], f32, kind="ExternalInput").ap()
    s = nc.dram_tensor("s", [P, FREE], out_dt, kind="ExternalOutput").ap()

    with tile.TileContext(nc) as tc:
        with (
            tc.tile_pool(name="state", bufs=1) as vpool,
            tc.tile_pool(name="sout", bufs=1) as spool,
        ):
            slabs = [
                vpool.tile([P, FREE_T], f32, tag=f"xs{t}", name=f"xs{t}")
                for t in range(T)
            ]
            qts = [
                vpool.tile([P, W], f32, tag=f"q{b}", name=f"q{b}")
                for b in range(BPC)
            ]
            outs = [
                spool.tile([P, T * W], out_dt, tag=f"s{b}", name=f"sout{b}")
                for b in range(BPC)
            ]
            biases = vpool.tile([P, T], f32, tag="bias", name="bias")
            for t in range(T):
                bias = float(np.nextafter(np.float32(-(1 << t)), np.float32(0)))
                nc.gpsimd.memset(biases[:, t:t + 1], bias)

            # plain loads, 1 MiB chunks, all on the sync HWDGE ring --
            # concurrent DMA on the scalar ring measured a ~20% slowdown of
            # every compute op.  t2 for chains 0/1 arrives via accum-DMAs;
            # load order tracks when each chunk is consumed.
            # t0 rides the scalar ring: it is idle during the ramp (before
            # any compute is running) so the dual-ring slowdown cannot bite,
            # and the sync ring starts on t1 immediately.  b0 is split in
            # half so the first STT starts ~2us earlier.
            nc.scalar.dma_start(slabs[0][:, :W // 2], x[:, :W // 2])
            nc.scalar.dma_start(slabs[0][:, W // 2:W], x[:, W // 2:W])
            for b in range(1, BPC):
                nc.scalar.dma_start(
                    slabs[0][:, b * W:(b + 1) * W], x[:, b * W:(b + 1) * W]
                )
            load_order = (
                [(1, b) for b in range(BPC)]
                + [(2, 2), (2, 3), (3, 2), (3, 3), (3, 0), (3, 1)]
            )
            for t, b in load_order:
                lo = t * FREE_T + b * W
                nc.sync.dma_start(
                    slabs[t][:, b * W:(b + 1) * W], x[:, lo:lo + W]
                )

            def state(b, t):
                return slabs[t][:, b * W:(b + 1) * W]

            def sign(b, t):
                nc.scalar.activation(
                    outs[b][:, t * W:(t + 1) * W], state(b, t), Act.Sign,
                    bias=biases[:, t:t + 1],
                )

            def store(b, t0_, t1_):
                nc.sync.dma_start(
                    s[:, b * T * W + t0_ * W:b * T * W + t1_ * W],
                    outs[b][:, t0_ * W:t1_ * W],
                )

            # prefetch the ACT Sign table during the load ramp
            nc.scalar.activation(
                outs[0][:, :1], biases[:, :1], Act.Sign, bias=biases[:, :1]
            )

            # t0: spikes + phase-batched STTs (arrival-paced; b0 halved to
            # match its split load)
            for b in range(BPC):
                sign(b, 0)
            for lo, hi in ((0, W // 2), (W // 2, W)):
                p = slabs[0][:, lo:hi]
                nc.vector.scalar_tensor_tensor(
                    qts[0][:, lo:hi], p, 1.0, p, Alu.is_lt, Alu.mult
                )
            for b in range(1, BPC):
                nc.vector.scalar_tensor_tensor(
                    qts[b][:], state(b, 0), 1.0, state(b, 0),
                    Alu.is_lt, Alu.mult,
                )
            # t1 per chain: [add(r1); STT(q1)].  Chains 0/1 then fire their
            # t2 accum-DMA immediately (13.5us+ completion latency hides
            # behind the rest of the DVE work); chains 2/3 do t2 on DVE.
            for b in range(BPC):
                p = state(b, 1)
                nc.vector.tensor_tensor(p, p, qts[b][:], Alu.add)
                nxt = state(b, 2)
                if b < 2:
                    nc.vector.scalar_tensor_tensor(
                        nxt, p, 2.0, p, Alu.is_lt, Alu.mult
                    )
                    nc.gpsimd.dma_start(
                        nxt,
                        x[:, 2 * FREE_T + b * W:2 * FREE_T + (b + 1) * W],
                        accum_op=Alu.add,
                    )
                else:
                    nc.vector.scalar_tensor_tensor(
                        qts[b][:], p, 2.0, p, Alu.is_lt, Alu.mult
                    )
                sign(b, 1)
                store(b, 0, 2)
            # t2/t3: DVE chains 2/3 first (their adds run while the chain
            # 0/1 accums complete), then the accum chains.  The last chain
            # runs its final timestep in halves so the closing
            # sign+store pipeline, shortening the tail.
            for b in (2, 3, 0, 1):
                p = state(b, 2)
                if b >= 2:
                    nc.vector.tensor_tensor(p, p, qts[b][:], Alu.add)
                sign(b, 2)
                store(b, 2, 3)
                nxt = state(b, 3)
                if b != 1:
                    nc.vector.scalar_tensor_tensor(
                        qts[b][:], p, 4.0, p, Alu.is_lt, Alu.mult
                    )
                    nc.vector.tensor_tensor(nxt, nxt, qts[b][:], Alu.add)
                    sign(b, 3)
                    store(b, 3, 4)
                else:
                    for lo, hi in ((0, W // 2), (W // 2, W)):
                        ph, qh = p[:, lo:hi], qts[b][:, lo:hi]
                        nc.vector.scalar_tensor_tensor(
                            qh, ph, 4.0, ph, Alu.is_lt, Alu.mult
                        )
                        nc.vector.tensor_tensor(
                            nxt[:, lo:hi], nxt[:, lo:hi], qh, Alu.add
                        )
                        nc.scalar.activation(
                            outs[b][:, 3 * W + lo:3 * W + hi], nxt[:, lo:hi],
                            Act.Sign, bias=biases[:, 3:4],
                        )
                        nc.sync.dma_start(
                            s[:, b * T * W + 3 * W + lo:b * T * W + 3 * W + hi],
                            outs[b][:, 3 * W + lo:3 * W + hi],
                        )
    nc.compile()
    return nc


def _shard(x):
    # x: (128, 256, 32, 32) f32 -> list of 8 per-core [128, 32768] arrays,
    # timestep t pre-scaled by 2^t (exact in fp32)
    xr = np.ascontiguousarray(x).reshape(T, BP, SLICE)
    tscale = (2.0 ** np.arange(T, dtype=np.float32)).astype(np.float32)
    shards = []
    for k in range(NCORES):
        xk = xr[:, k * BPC:(k + 1) * BPC, :].reshape(T, BPC, P, W)
        xk = xk * tscale[:, None, None, None]
        xk = xk.transpose(2, 0, 1, 3).reshape(P, FREE)
        shards.append(np.asarray(xk, dtype=np.float32))
    return shards


def _unshard(parts):
    # parts: 8 per-core [128, 32768] arrays (fp8 sign values, b-major)
    # -> (128,256,32,32) f32 spikes; spike iff stored value > 0
    out = np.empty((T, BP, SLICE), dtype=np.float32)
    for k, sk in enumerate(parts):
        sk = (np.asarray(sk).astype(np.float32) > 0).astype(np.float32)
        sk = sk.reshape(P, BPC, T, W)
        out[:, k * BPC:(k + 1) * BPC, :] = (
            sk.transpose(2, 1, 0, 3).reshape(T, BPC, SLICE)
        )
    return out.reshape(T * BP, 256, 32, 32)


def _in_maps(x):
    return [{"x": sk} for sk in _shard(np.asarray(x, dtype=np.float32))]


def kernel(x):
    from concourse.bass_utils import run_bass_kernel_spmd

    if "nc" not in _cache:
        _cache["nc"] = _build_program()
    nc = _cache["nc"]

    res = run_bass_kernel_spmd(nc, _in_maps(x), list(range(NCORES)))
    return _unshard([res.results[k]["s"] for k in range(NCORES)])
